# revision 1
# baseline (speedup 1.0000x reference)
"""Causal multi-head attention block on 8 TRN2 NeuronCores.

Sharding: tensor-parallel over heads (2 heads/core, both batches) for the
QKV projection + attention; an on-device AllToAll re-shards to
sequence-parallel for the output projection (Megatron-style). Matmuls run in
float32r (full PE rate, ~1.5e-4 rel err). Scores use zero-padded per-head
K^T copies so they run as full K=128 matmuls (half-height K=64 row-tiled
matmuls measured 1.8x slower per-op on HW).

Self-contained: hardcodes all shapes from the problem spec.
"""

import numpy as np
from contextlib import ExitStack

import concourse.bass as bass
import concourse.tile as tile
from concourse import bacc, mybir
from concourse.bass_utils import run_bass_kernel_spmd

F32R = mybir.dt.float32r
F32 = mybir.dt.float32
AF = mybir.ActivationFunctionType

B, T, C, H, HD = 2, 2048, 1024, 16, 64
NCORES = 8
BT = B * T            # 4096 global rows
TQ = 512              # q-chunk width
KT = 128              # k-tile height
NJ = T // TQ          # 4 q-chunks per batch (= per core)
NKK = T // KT         # 16 k-tiles per batch
NCT = C // 128        # 8 contraction tiles for projections
NTC = BT // TQ        # 8 global t-chunks
TSL = BT // NCORES    # 512 rows of final output per core
SPOOL_BUFS = 3
PO_BUFS = 2
QUICK_EVICT = True
ATTN_INTERLEAVE = False
XT_SHAPE = [NCT, NTC, 128, TQ]  # tile-contiguous full x^T


def build(with_collective=True):
    nc = bacc.Bacc(None, target_bir_lowering=False)

    xt = nc.dram_tensor("xt", XT_SHAPE, F32R, kind="ExternalInput")
    wqkv = nc.dram_tensor("wqkv", [C, 3 * 128], F32R, kind="ExternalInput")
    bqkv = nc.dram_tensor("bqkv", [128, 3], F32, kind="ExternalInput")
    wout = nc.dram_tensor("wout", [C, C], F32R, kind="ExternalInput")
    bout = nc.dram_tensor("bout", [128, C], F32, kind="ExternalInput")
    out = nc.dram_tensor("out", [TSL, C], F32, kind="ExternalOutput")

    ident_d = nc.dram_tensor("ident", [128, 128], F32R, kind="ExternalInput")
    ones_d = nc.dram_tensor("ones", [128, 64], F32R, kind="ExternalInput")
    zeros_d = nc.dram_tensor("zeros", [64, TQ], F32R, kind="ExternalInput")
    a2a_in = nc.dram_tensor("a2a_in", [NCORES, 128, TQ], F32R)
    a2a_out = nc.dram_tensor("a2a_out", [NCORES, 128, TQ], F32R)

    with tile.TileContext(nc) as tc:
        _emit(nc, tc, xt, wqkv, bqkv, wout, bout, out, a2a_in, a2a_out,
              ident_d, ones_d, zeros_d, with_collective)
    nc.compile()
    return nc


def _emit(nc, tc, xt, wqkv, bqkv, wout, bout, out, a2a_in, a2a_out,
          ident_d, ones_d, zeros_d, with_collective, prefetch_wout="mid",
          trunc=None):
    with ExitStack() as ctx:
        persist = ctx.enter_context(tc.tile_pool(name="persist", bufs=1))

        # persistent SBUF tensors, indexed by batch b (the core owns the
        # same 2 heads in both batches).
        qts = [persist.tile([128, T], F32R, tag=f"qt{p}", name=f"qt{p}")
               for p in range(2)]
        # zero-padded per-head K^T (head h lives in rows 64*(h%2);
        # the other 64 rows are zero so scores run as full K=128 matmuls)
        kts = [persist.tile([128, T], F32R, tag=f"kt{h}", name=f"kt{h}")
               for h in range(4)]
        va = persist.tile([128, 2, NKK, 192], F32R, tag="va")  # [V_e|ones|V_o]
        wsb = persist.tile([128, NCT, 384], F32R, tag="wsb")
        bsb = persist.tile([128, 3], F32, tag="bsb")
        ident = persist.tile([128, 128], F32R, tag="ident")
        wosb = persist.tile([128, NCT, C], F32R, tag="wo")
        bosb = persist.tile([128, C], F32, tag="bo")

        nc.sync.dma_start(wsb[:], wqkv[:].rearrange("(n p) c -> p n c", p=128))
        nc.sync.dma_start(bsb[:], bqkv[:])
        if prefetch_wout == "early":
            nc.sync.dma_start(wosb[:], wout[:].rearrange("(n p) c -> p n c", p=128))
            nc.sync.dma_start(bosb[:], bout[:])

        # zero padding of kts + identity + VA ones (host constants)
        for h in range(4):
            dead = slice(64, 128) if h % 2 == 0 else slice(0, 64)
            for z in range(NJ):
                nc.sync.dma_start(kts[h][dead, TQ * z:TQ * (z + 1)], zeros_d[:])
        nc.sync.dma_start(ident[:], ident_d[:])
        for p0 in range(2):
            for tt0 in range(NKK):
                nc.sync.dma_start(va[:, p0, tt0, 64:128], ones_d[:])

        # ---- phase 1: QKV^T projection (+ V transpose into VA) ----
        with (
            tc.tile_pool(name="xtile", bufs=24) as xpool,
            tc.tile_pool(name="pp", bufs=6, space="PSUM") as pp,
            tc.tile_pool(name="vtile", bufs=3) as vpool,
            tc.tile_pool(name="ptr", bufs=2, space="PSUM") as ptr,
        ):
            for tc0 in range(NTC):
                b, jloc = tc0 // NJ, tc0 % NJ
                xts = []
                for kc in range(NCT):
                    xtile = xpool.tile([128, TQ], F32R, tag="x",
                                       name=f"x{tc0}_{kc}")
                    nc.sync.dma_start(xtile[:], xt[kc, tc0])
                    xts.append(xtile)
                for g in range(3):
                    gcol = 128 * g
                    ps = pp.tile([128, TQ], F32, tag="pp", name=f"pp{tc0}_{g}")
                    for kc in range(NCT):
                        nc.tensor.matmul(ps[:], wsb[:, kc, gcol:gcol + 128],
                                         xts[kc][:],
                                         start=(kc == 0), stop=(kc == NCT - 1))
                    chunk = slice(TQ * jloc, TQ * (jloc + 1))
                    if g == 0:        # Q^T of batch b
                        nc.vector.tensor_scalar_add(qts[b][:, chunk], ps[:],
                                                    bsb[:, 0:1])
                    elif g == 1:      # K^T of batch b, split per head
                        nc.vector.tensor_scalar_add(
                            kts[2 * b][0:64, chunk], ps[0:64, :],
                            bsb[0:64, 1:2])
                        nc.vector.tensor_scalar_add(
                            kts[2 * b + 1][64:128, chunk], ps[64:128, :],
                            bsb[64:128, 1:2])
                    else:             # V of batch b -> transpose into VA
                        vtile = vpool.tile([128, TQ], F32R, tag="v",
                                           name=f"v{tc0}")
                        nc.vector.tensor_scalar_add(vtile[:], ps[:],
                                                    bsb[:, 2:3])
                        for q in range(4):
                            tt = jloc * 4 + q   # k-tile index in batch b
                            pst = ptr.tile([128, 128], F32R, tag="pt",
                                           name=f"pt{tc0}_{q}")
                            nc.tensor.matmul(pst[:],
                                             vtile[:, 128 * q:128 * (q + 1)],
                                             ident[:], is_transpose=True)
                            nc.vector.tensor_copy(va[:, b, tt, 0:64],
                                                  pst[:, 0:64])
                            nc.vector.tensor_copy(va[:, b, tt, 128:192],
                                                  pst[:, 64:128])

        if trunc == "proj":
            with tc.tile_pool(name="dumo", bufs=1) as dpool:
                d = dpool.tile([128, TQ], F32, tag="d")
                nc.vector.tensor_copy(d[:], qts[0][0:128, 0:TQ].bitcast(F32))
                nc.sync.dma_start(out[0:128, 0:TQ], d[:])
            return

        # ---- phase 2: attention (p = batch index) ----
        with (
            tc.tile_pool(name="psc", bufs=SPOOL_BUFS, space="PSUM") as spool,
            tc.tile_pool(name="po", bufs=PO_BUFS, space="PSUM") as opool,
            tc.tile_pool(name="ptp", bufs=6) as ptpool,
            tc.tile_pool(name="yt", bufs=3) as ytpool,
            tc.tile_pool(name="rt", bufs=3) as rtpool,
            tc.tile_pool(name="oe", bufs=4) as oepool,
        ):
            order = ([(p, j) for p in range(2) for j in range(NJ)]
                     if not ATTN_INTERLEAVE else
                     [(p, j) for j in range(NJ) for p in range(2)])
            for oi, (p, j) in enumerate(order):
                    if oi == 4 and prefetch_wout == "mid":
                        nc.sync.dma_start(wosb[:],
                                          wout[:].rearrange("(n p) c -> p n c",
                                                            p=128))
                        nc.sync.dma_start(bosb[:], bout[:])
                    nkk = 4 * (j + 1)
                    po = [opool.tile([128, TQ], F32, tag="po",
                                     name=f"po{p}_{j}_{h}") for h in range(2)]
                    for kk in range(nkk):
                        ps_s = spool.tile([128, 2 * TQ], F32, tag="s",
                                          name=f"s{p}_{j}_{kk}")
                        for h2 in range(2):
                            nc.tensor.matmul(
                                ps_s[:, TQ * h2:TQ * (h2 + 1)],
                                kts[2 * p + h2][:, KT * kk:KT * (kk + 1)],
                                qts[p][:, TQ * j:TQ * (j + 1)],
                                start=True, stop=True)
                        pt = ptpool.tile([128, 2 * TQ], F32R, tag="pt",
                                         name=f"p{p}_{j}_{kk}")
                        nc.scalar.activation(pt[:], ps_s[:], AF.Exp)
                        o = max(kk - 4 * j, 0)  # suffix offset (diag tiles)
                        if kk >= 4 * j:
                            for h2 in range(2):
                                lo = TQ * h2 + KT * o
                                # aligned triangle: keep qf' >= r
                                nc.gpsimd.affine_select(
                                    out=pt[:, lo:TQ * (h2 + 1)],
                                    in_=pt[:, lo:TQ * (h2 + 1)],
                                    compare_op=mybir.AluOpType.is_ge,
                                    fill=0.0, base=0,
                                    pattern=[[1, TQ - KT * o]],
                                    channel_multiplier=-1)
                        for h2 in range(2):
                            vs = slice(0, 128) if h2 == 0 else slice(64, 192)
                            nc.tensor.matmul(
                                po[h2][:, KT * o:TQ],
                                va[:, p, kk, vs],
                                pt[:, TQ * h2 + KT * o:TQ * (h2 + 1)],
                                start=(kk == 0), stop=(kk == nkk - 1))
                    # normalize: h0 sums in rows 64:128, h1 sums in rows 0:64
                    yt = ytpool.tile([128, TQ], F32R, tag="yt", name=f"y{p}_{j}")
                    rt = rtpool.tile([128, TQ], F32, tag="rt", name=f"r{p}_{j}")
                    if QUICK_EVICT:
                        # copy psum->sbuf fast so the accumulator banks free
                        # for the next q-chunk before the recip/mul run
                        oes = [oepool.tile([128, TQ], F32, tag="oe",
                                           name=f"oe{p}_{j}_{h}")
                               for h in range(2)]
                        nc.vector.tensor_copy(oes[0][:], po[0][:])
                        nc.vector.tensor_copy(oes[1][:], po[1][:])
                        src0, src1 = oes[0], oes[1]
                    else:
                        src0, src1 = po[0], po[1]
                    nc.vector.reciprocal(rt[0:64, :], src0[64:128, :])
                    nc.vector.tensor_mul(yt[0:64, :], src0[0:64, :], rt[0:64, :])
                    nc.vector.reciprocal(rt[64:128, :], src1[0:64, :])
                    nc.vector.tensor_mul(yt[64:128, :], src1[64:128, :],
                                         rt[64:128, :])
                    nc.sync.dma_start(a2a_in[p * NJ + j, :, :], yt[:])

        if trunc == "attn":
            with tc.tile_pool(name="dumo2", bufs=1) as dpool2:
                d2 = dpool2.tile([128, TQ], F32, tag="d2")
                nc.vector.tensor_copy(d2[:], qts[0][0:128, 0:TQ].bitcast(F32))
                nc.sync.dma_start(out[0:128, 0:TQ], d2[:])
            return

        # ---- phase 3: all-to-all (head-sharded -> t-sharded) ----
        if with_collective is True:
            nc.gpsimd.collective_compute(
                "AllToAll", mybir.AluOpType.bypass,
                replica_groups=[list(range(NCORES))],
                ins=[a2a_in[:]], outs=[a2a_out[:]])
        elif with_collective is False:
            nc.sync.dma_start(a2a_out[:], a2a_in[:])
        # else (None): timing mode — caller aliases a2a_out to a2a_in

        # ---- phase 4: output projection (rows TSL per core) ----
        with (
            tc.tile_pool(name="yts", bufs=1) as ytspool,
            tc.tile_pool(name="pout", bufs=4, space="PSUM") as poutp,
            tc.tile_pool(name="osb", bufs=4) as osbpool,
        ):
            yts = ytspool.tile([128, NCT, TQ], F32R, tag="yts")
            for cc in range(NCT):
                nc.sync.dma_start(yts[:, cc, :], a2a_out[cc, :, :])

            for tt in range(TSL // 128):
                pos = [poutp.tile([128, TQ], F32, tag="pout",
                                  name=f"pos{tt}_{h}") for h in range(2)]
                for cc in range(NCT):
                    for n in range(2):
                        nc.tensor.matmul(
                            pos[n][:], yts[:, cc, 128 * tt:128 * (tt + 1)],
                            wosb[:, cc, TQ * n:TQ * (n + 1)],
                            start=(cc == 0), stop=(cc == NCT - 1))
                for n in range(2):
                    osb = osbpool.tile([128, TQ], F32, tag="osb")
                    nc.vector.tensor_add(osb[:], pos[n][:],
                                         bosb[:, TQ * n:TQ * (n + 1)])
                    nc.sync.dma_start(
                        out[128 * tt:128 * (tt + 1), TQ * n:TQ * (n + 1)],
                        osb[:])


def make_core_inputs(x, w_qkv, b_qkv, w_out, b_out):
    """Host-side shard/transform. Returns list of per-core input dicts."""
    x = np.asarray(x, np.float32)
    w_qkv = np.asarray(w_qkv, np.float32)
    b_qkv = np.asarray(b_qkv, np.float32)
    w_out = np.asarray(w_out, np.float32)
    b_out = np.asarray(b_out, np.float32)

    bout_rep = np.ascontiguousarray(np.broadcast_to(b_out, (128, C)))
    # tile-contiguous x^T: xt[kc, tc0, p, q] = x_flat[TQ*tc0+q, 128*kc+p]
    xt = np.ascontiguousarray(
        x.reshape(NTC, TQ, NCT, 128).transpose(2, 0, 3, 1))
    in_maps = []
    for c in range(NCORES):
        s = slice(128 * c, 128 * (c + 1))
        wq = w_qkv[:, :C][:, s] * 0.125
        wk = w_qkv[:, C:2 * C][:, s]
        wv = w_qkv[:, 2 * C:][:, s]
        wc = np.ascontiguousarray(np.concatenate([wq, wk, wv], axis=1))
        bc3 = np.ascontiguousarray(
            np.stack([b_qkv[:C][s] * 0.125, b_qkv[C:2 * C][s],
                      b_qkv[2 * C:][s]], axis=1))
        in_maps.append({
            "xt": xt, "wqkv": wc, "bqkv": bc3,
            "wout": w_out, "bout": bout_rep,
            "ident": np.eye(128, dtype=np.float32),
            "ones": np.ones((128, 64), np.float32),
            "zeros": np.zeros((64, TQ), np.float32),
        })
    return in_maps


_NC_CACHE = {}


def _make_cached_runner(nc):
    """Jit the SPMD executable once; subsequent calls only re-upload inputs."""
    import jax
    from jax.sharding import Mesh, PartitionSpec
    from jax.experimental.shard_map import shard_map
    from concourse.bass2jax import _bass_exec_p, install_neuronx_cc_hook

    install_neuronx_cc_hook()
    in_names, out_names, out_avals = [], [], []
    for alloc in nc.m.functions[0].allocations:
        if not isinstance(alloc, mybir.MemoryLocationSet):
            continue
        name = alloc.memorylocations[0].name
        if alloc.kind == "ExternalInput":
            in_names.append(name)
        elif alloc.kind == "ExternalOutput":
            out_names.append(name)
            out_avals.append(jax.core.ShapedArray(
                tuple(alloc.tensor_shape), mybir.dt.np(alloc.dtype)))
    n_params = len(in_names)
    all_in = list(in_names) + list(out_names)

    def _body(*args):
        outs = _bass_exec_p.bind(
            *args, out_avals=tuple(out_avals), in_names=tuple(all_in),
            out_names=tuple(out_names), lowering_input_output_aliases=(),
            sim_require_finite=True, sim_require_nnan=True, nc=nc)
        return tuple(outs)

    devices = jax.devices()[:NCORES]
    mesh = Mesh(np.asarray(devices), ("core",))
    spec = PartitionSpec("core")
    sharded = jax.jit(
        shard_map(_body, mesh=mesh,
                  in_specs=(spec,) * (n_params + len(out_names)),
                  out_specs=(spec,) * len(out_names), check_rep=False),
        keep_unused=True)
    zeros = [np.zeros((NCORES * a.shape[0], *a.shape[1:]), a.dtype)
             for a in out_avals]

    def run(in_maps):
        concat = [np.concatenate([np.asarray(m[nm]) for m in in_maps], axis=0)
                  for nm in in_names]
        outs = sharded(*concat, *zeros)
        return {nm: np.asarray(outs[i]) for i, nm in enumerate(out_names)}

    return run


def kernel(x, w_qkv, b_qkv, w_out, b_out):
    in_maps = make_core_inputs(x, w_qkv, b_qkv, w_out, b_out)
    if "nc" not in _NC_CACHE:
        _NC_CACHE["nc"] = build()
    nc = _NC_CACHE["nc"]
    try:
        if "run" not in _NC_CACHE:
            _NC_CACHE["run"] = _make_cached_runner(nc)
        outs = _NC_CACHE["run"](in_maps)
        full = outs["out"].reshape(NCORES * TSL, C)
    except Exception:
        res = run_bass_kernel_spmd(nc, in_maps, core_ids=list(range(NCORES)))
        full = np.concatenate([res.results[c]["out"] for c in range(NCORES)],
                              axis=0)
    return full.reshape(B, T, C)



# revision 4
# speedup vs baseline: 3.3332x; 3.3332x over previous
"""Causal multi-head attention block on 8 TRN2 NeuronCores.

Sharding: tensor-parallel over heads (2 heads/core, both batches) for the
QKV projection + attention; an on-device AllToAll re-shards to
sequence-parallel for the output projection (Megatron-style).

v2 structure: QKV projection and attention are fused into one software
pipeline over (batch, q-chunk) — projection of chunk c+1 is emitted
interleaved with attention of chunk c so PE and ACT overlap instead of
running as serial phases.  Init DMAs are batched (one DMA per x chunk via a
4-dim AP, one strided DMA for the VA ones block).  The V projection bias is
folded into the output bias on the host (softmax rows sum to 1), so V needs
no on-device bias add.  Diagonal score tiles truncate the matmul, exp, and
mask to the causal region.

Matmuls run in float32r (full PE rate at N>=256, ~1.5e-4 rel err).  Scores
use zero-padded per-head K^T copies so they run as full K=128 matmuls
(half-height K=64 row-tiled matmuls measured 1.8x slower per-op on HW).

Self-contained: hardcodes all shapes from the problem spec.
"""

import numpy as np
from contextlib import ExitStack

import concourse.bass as bass
import concourse.tile as tile
from concourse import bacc, mybir
from concourse.bass_utils import run_bass_kernel_spmd

F32R = mybir.dt.float32r
F32 = mybir.dt.float32
AF = mybir.ActivationFunctionType

B, T, C, H, HD = 2, 2048, 1024, 16, 64
NCORES = 8
BT = B * T            # 4096 global rows
TQ = 512              # q-chunk width
KT = 128              # k-tile height
NJ = T // TQ          # 4 q-chunks per batch (= per core)
NKK = T // KT         # 16 k-tiles per batch
NCT = C // 128        # 8 contraction tiles for projections
NTC = BT // TQ        # 8 global t-chunks
TSL = BT // NCORES    # 512 rows of final output per core
XT_SHAPE = [NCT, NTC, 128, TQ]  # tile-contiguous full x^T


def declare_io(nc):
    """DRAM tensors shared by build() and the timing loop builder."""
    d = {}
    d["xt"] = nc.dram_tensor("xt", XT_SHAPE, F32R, kind="ExternalInput")
    d["wqkv"] = nc.dram_tensor("wqkv", [C, 3 * 128], F32R,
                               kind="ExternalInput")
    d["bqkv"] = nc.dram_tensor("bqkv", [128, 3], F32, kind="ExternalInput")
    d["wout"] = nc.dram_tensor("wout", [C, C], F32R, kind="ExternalInput")
    d["bout"] = nc.dram_tensor("bout", [128, C], F32, kind="ExternalInput")
    d["out"] = nc.dram_tensor("out", [TSL, C], F32, kind="ExternalOutput")
    d["ident"] = nc.dram_tensor("ident", [128, 128], F32R,
                                kind="ExternalInput")
    d["vaones"] = nc.dram_tensor("vaones", [128, 2, NKK, 64], F32R,
                                 kind="ExternalInput")
    d["zeros"] = nc.dram_tensor("zeros", [64, T], F32R, kind="ExternalInput")
    return d


def build(with_collective=True):
    nc = bacc.Bacc(None, target_bir_lowering=False)
    d = declare_io(nc)
    a2a_in = nc.dram_tensor("a2a_in", [NCORES, 128, TQ], F32R)
    if with_collective is None:
        a2a_out = a2a_in
    else:
        a2a_out = nc.dram_tensor("a2a_out", [NCORES, 128, TQ], F32R)
    with tile.TileContext(nc) as tc:
        _emit(nc, tc, d, a2a_in, a2a_out, with_collective)
    nc.compile()
    return nc


def _emit(nc, tc, d, a2a_in, a2a_out, with_collective, trunc=None):
    xt, wqkv, bqkv = d["xt"], d["wqkv"], d["bqkv"]
    wout, bout, out = d["wout"], d["bout"], d["out"]

    with ExitStack() as ctx:
        persist = ctx.enter_context(tc.tile_pool(name="persist", bufs=1))

        # persistent SBUF tensors, indexed by batch b (the core owns the
        # same 2 heads in both batches).
        qts = [persist.tile([128, T], F32R, tag=f"qt{p}", name=f"qt{p}")
               for p in range(2)]
        # zero-padded per-head K^T (head h lives in rows 64*(h%2);
        # the other 64 rows are zero so scores run as full K=128 matmuls)
        kts = [persist.tile([128, T], F32R, tag=f"kt{h}", name=f"kt{h}")
               for h in range(4)]
        va = persist.tile([128, 2, NKK, 192], F32R, tag="va")  # [V_e|ones|V_o]
        wsb = persist.tile([128, NCT, 384], F32R, tag="wsb")
        bsb = persist.tile([128, 3], F32, tag="bsb")
        ident = persist.tile([128, 128], F32R, tag="ident")
        wosb = persist.tile([128, NCT, C], F32R, tag="wo")
        bosb = persist.tile([128, C], F32, tag="bo")

        nc.sync.dma_start(wsb[:], wqkv[:].rearrange("(n p) c -> p n c", p=128))
        nc.sync.dma_start(bsb[:], bqkv[:])
        nc.sync.dma_start(ident[:], d["ident"][:])
        # zero the dead half of each per-head K^T (one DMA per head)
        for h in range(4):
            dead = slice(64, 128) if h % 2 == 0 else slice(0, 64)
            nc.sync.dma_start(kts[h][dead, :], d["zeros"][:])
        # ones block of VA in one strided DMA
        nc.sync.dma_start(va[:, :, :, 64:128], d["vaones"][:])

        pipe = ctx.enter_context(ExitStack())
        pools = {}
        pools["x"] = pipe.enter_context(tc.tile_pool(name="xtile", bufs=2))
        pools["pp"] = pipe.enter_context(
            tc.tile_pool(name="pp", bufs=1, space="PSUM"))
        pools["ptr"] = pipe.enter_context(
            tc.tile_pool(name="ptr", bufs=1, space="PSUM"))
        pools["v"] = pipe.enter_context(tc.tile_pool(name="vtile", bufs=2))
        pools["s"] = pipe.enter_context(
            tc.tile_pool(name="psc", bufs=2, space="PSUM"))
        pools["o"] = pipe.enter_context(
            tc.tile_pool(name="po", bufs=2, space="PSUM"))
        pools["pt"] = pipe.enter_context(tc.tile_pool(name="ptp", bufs=5))
        pools["yt"] = pipe.enter_context(tc.tile_pool(name="yt", bufs=2))
        pools["rt"] = pipe.enter_context(tc.tile_pool(name="rt", bufs=2))
        pools["oe"] = pipe.enter_context(tc.tile_pool(name="oe", bufs=2))

        def proj_steps(tc0):
            """Generator: emit projection of chunk tc0, yielding between
            units so attention of the previous chunk can interleave."""
            b, jloc = divmod(tc0, NJ)
            chunk = slice(TQ * jloc, TQ * (jloc + 1))
            xtile = pools["x"].tile([128, NCT, TQ], F32R, tag="x",
                                    name=f"x{tc0}")
            nc.sync.dma_start(
                xtile[:], xt[:, tc0].rearrange("k p q -> p k q"))
            yield
            for g in range(2):          # 0 = Q^T, 1 = K^T
                ps = pools["pp"].tile([128, TQ], F32, tag="pp",
                                      name=f"pp{tc0}_{g}")
                for kc in range(NCT):
                    nc.tensor.matmul(ps[:], wsb[:, kc, 128 * g:128 * (g + 1)],
                                     xtile[:, kc, :],
                                     start=(kc == 0), stop=(kc == NCT - 1))
                    if kc % 4 == 3:
                        yield
                if g == 0:
                    nc.vector.tensor_scalar_add(qts[b][:, chunk], ps[:],
                                                bsb[:, 0:1])
                else:
                    nc.vector.tensor_scalar_add(
                        kts[2 * b][0:64, chunk], ps[0:64, :], bsb[0:64, 1:2])
                    nc.vector.tensor_scalar_add(
                        kts[2 * b + 1][64:128, chunk], ps[64:128, :],
                        bsb[64:128, 1:2])
                yield
            # V^T (bias folded into bout on host), then PE transpose into VA
            psv = pools["pp"].tile([128, TQ], F32, tag="pp", name=f"ppv{tc0}")
            for kc in range(NCT):
                nc.tensor.matmul(psv[:], wsb[:, kc, 256:384], xtile[:, kc, :],
                                 start=(kc == 0), stop=(kc == NCT - 1))
                if kc % 4 == 3:
                    yield
            vtile = pools["v"].tile([128, TQ], F32R, tag="v", name=f"v{tc0}")
            nc.vector.tensor_copy(vtile[:], psv[:])
            yield
            for q in range(4):
                tt = jloc * 4 + q       # k-tile index in batch b
                pst = pools["ptr"].tile([128, 128], F32R, tag="pt2",
                                        name=f"pt2_{tc0}_{q}")
                nc.tensor.matmul(pst[:], vtile[:, 128 * q:128 * (q + 1)],
                                 ident[:], is_transpose=True)
                nc.vector.tensor_copy(va[:, b, tt, 0:64], pst[:, 0:64])
                nc.vector.tensor_copy(va[:, b, tt, 128:192], pst[:, 64:128])
                yield

        def attend_steps(p, j):
            """Generator: emit attention for q-chunk (p, j), yielding after
            each k-tile."""
            nkk = 4 * (j + 1)
            po = [pools["o"].tile([128, TQ], F32, tag="po",
                                  name=f"po{p}_{j}_{h}") for h in range(2)]
            for kk in range(nkk):
                o = max(kk - 4 * j, 0)  # suffix offset (diagonal tiles)
                lo = KT * o
                ps_s = pools["s"].tile([128, 2, TQ], F32, tag="s",
                                       name=f"s{p}_{j}_{kk}")
                for h2 in range(2):
                    nc.tensor.matmul(
                        ps_s[:, h2, lo:],
                        kts[2 * p + h2][:, KT * kk:KT * (kk + 1)],
                        qts[p][:, TQ * j + lo:TQ * (j + 1)],
                        start=True, stop=True)
                pt = pools["pt"].tile([128, 2, TQ], F32R, tag="pt",
                                      name=f"p{p}_{j}_{kk}")
                nc.scalar.activation(pt[:, :, lo:], ps_s[:, :, lo:], AF.Exp)
                if kk >= 4 * j:
                    for h2 in range(2):
                        # aligned triangle: keep qf' >= r
                        nc.gpsimd.affine_select(
                            out=pt[:, h2, lo:],
                            in_=pt[:, h2, lo:],
                            compare_op=mybir.AluOpType.is_ge,
                            fill=0.0, base=0,
                            pattern=[[1, TQ - lo]],
                            channel_multiplier=-1)
                for h2 in range(2):
                    vs = slice(0, 128) if h2 == 0 else slice(64, 192)
                    nc.tensor.matmul(
                        po[h2][:, lo:], va[:, p, kk, vs], pt[:, h2, lo:],
                        start=(kk == 0), stop=(kk == nkk - 1))
                yield
            # normalize: h0 sums in rows 64:128, h1 sums in rows 0:64
            # (copy psum->sbuf fast so the accumulator banks free early)
            oes = [pools["oe"].tile([128, TQ], F32, tag="oe",
                                    name=f"oe{p}_{j}_{h}") for h in range(2)]
            nc.vector.tensor_copy(oes[0][:], po[0][:])
            nc.vector.tensor_copy(oes[1][:], po[1][:])
            yt = pools["yt"].tile([128, TQ], F32R, tag="yt", name=f"y{p}_{j}")
            rt = pools["rt"].tile([128, TQ], F32, tag="rt", name=f"r{p}_{j}")
            nc.vector.reciprocal(rt[0:64, :], oes[0][64:128, :])
            nc.vector.tensor_mul(yt[0:64, :], oes[0][0:64, :], rt[0:64, :])
            nc.vector.reciprocal(rt[64:128, :], oes[1][0:64, :])
            nc.vector.tensor_mul(yt[64:128, :], oes[1][64:128, :],
                                 rt[64:128, :])
            nc.sync.dma_start(a2a_in[p * NJ + j, :, :], yt[:])

        # ---- fused pipeline: proj(c+1) interleaved with attend(c) ----
        def drain(g):
            for _ in g:
                pass

        order = [(p, j) for p in range(2) for j in range(NJ)]
        drain_proj = trunc == "proj"
        if drain_proj:
            for idx in range(NTC):
                drain(proj_steps(idx))
        else:
            drain(proj_steps(0))
            for idx, (p, j) in enumerate(order):
                if idx == 4:
                    nc.sync.dma_start(
                        wosb[:], wout[:].rearrange("(n p) c -> p n c", p=128))
                    nc.sync.dma_start(bosb[:], bout[:])
                ag = attend_steps(p, j)
                pg = proj_steps(idx + 1) if idx + 1 < NTC else None
                nkk = 4 * (j + 1)
                np_est = 16
                acc = 0
                for i in range(nkk):
                    if next(ag, "end") == "end":
                        break
                    if pg is not None:
                        want = ((i + 1) * np_est) // nkk
                        while acc < want:
                            if next(pg, "end") == "end":
                                pg = None
                                break
                            acc += 1
                drain(ag)
                if pg is not None:
                    drain(pg)

        pipe.close()

        if trunc in ("proj", "attn"):
            with tc.tile_pool(name="dumo", bufs=1) as dpool:
                dm = dpool.tile([128, TQ], F32, tag="d")
                nc.vector.tensor_copy(dm[:], qts[0][0:128, 0:TQ].bitcast(F32))
                nc.sync.dma_start(out[0:128, 0:TQ], dm[:])
            return

        # ---- all-to-all (head-sharded -> t-sharded) ----
        if with_collective is True:
            nc.gpsimd.collective_compute(
                "AllToAll", mybir.AluOpType.bypass,
                replica_groups=[list(range(NCORES))],
                ins=[a2a_in[:]], outs=[a2a_out[:]])
        elif with_collective is False:
            nc.sync.dma_start(a2a_out[:], a2a_in[:])
        # else (None): timing mode — caller aliases a2a_out to a2a_in

        # ---- output projection (rows TSL per core) ----
        with (
            tc.tile_pool(name="yts", bufs=1) as ytspool,
            tc.tile_pool(name="pout", bufs=4, space="PSUM") as poutp,
            tc.tile_pool(name="osb", bufs=4) as osbpool,
        ):
            yts = ytspool.tile([128, NCT, TQ], F32R, tag="yts")
            for cc in range(NCT):
                nc.sync.dma_start(yts[:, cc, :], a2a_out[cc, :, :])

            for tt in range(TSL // 128):
                pos = [poutp.tile([128, TQ], F32, tag="pout",
                                  name=f"pos{tt}_{h}") for h in range(2)]
                for cc in range(NCT):
                    for n in range(2):
                        nc.tensor.matmul(
                            pos[n][:], yts[:, cc, 128 * tt:128 * (tt + 1)],
                            wosb[:, cc, TQ * n:TQ * (n + 1)],
                            start=(cc == 0), stop=(cc == NCT - 1))
                for n in range(2):
                    osb = osbpool.tile([128, TQ], F32, tag="osb")
                    nc.vector.tensor_add(osb[:], pos[n][:],
                                         bosb[:, TQ * n:TQ * (n + 1)])
                    nc.sync.dma_start(
                        out[128 * tt:128 * (tt + 1), TQ * n:TQ * (n + 1)],
                        osb[:])


def make_core_inputs(x, w_qkv, b_qkv, w_out, b_out):
    """Host-side shard/transform. Returns list of per-core input dicts."""
    x = np.asarray(x, np.float32)
    w_qkv = np.asarray(w_qkv, np.float32)
    b_qkv = np.asarray(b_qkv, np.float32)
    w_out = np.asarray(w_out, np.float32)
    b_out = np.asarray(b_out, np.float32)

    # softmax rows sum to 1, so the V bias contributes (b_v @ w_out) to
    # every output row — fold it into the output bias.
    b_eff = b_out + b_qkv[2 * C:] @ w_out
    bout_rep = np.ascontiguousarray(
        np.broadcast_to(b_eff.astype(np.float32), (128, C)))
    # tile-contiguous x^T: xt[kc, tc0, p, q] = x_flat[TQ*tc0+q, 128*kc+p]
    xt = np.ascontiguousarray(
        x.reshape(NTC, TQ, NCT, 128).transpose(2, 0, 3, 1))
    vaones = np.ones((128, 2, NKK, 64), np.float32)
    zeros = np.zeros((64, T), np.float32)
    ident = np.eye(128, dtype=np.float32)
    in_maps = []
    for c in range(NCORES):
        s = slice(128 * c, 128 * (c + 1))
        wq = w_qkv[:, :C][:, s] * 0.125
        wk = w_qkv[:, C:2 * C][:, s]
        wv = w_qkv[:, 2 * C:][:, s]
        wc = np.ascontiguousarray(np.concatenate([wq, wk, wv], axis=1))
        bc3 = np.ascontiguousarray(
            np.stack([b_qkv[:C][s] * 0.125, b_qkv[C:2 * C][s],
                      np.zeros(128, np.float32)], axis=1))
        in_maps.append({
            "xt": xt, "wqkv": wc, "bqkv": bc3,
            "wout": w_out, "bout": bout_rep,
            "ident": ident, "vaones": vaones, "zeros": zeros,
        })
    return in_maps


_NC_CACHE = {}


def kernel(x, w_qkv, b_qkv, w_out, b_out):
    in_maps = make_core_inputs(x, w_qkv, b_qkv, w_out, b_out)
    if "nc" not in _NC_CACHE:
        _NC_CACHE["nc"] = build()
    nc = _NC_CACHE["nc"]
    res = run_bass_kernel_spmd(nc, in_maps, core_ids=list(range(NCORES)))
    full = np.concatenate([res.results[c]["out"] for c in range(NCORES)],
                          axis=0)
    return full.reshape(B, T, C)


# revision 40
# speedup vs baseline: 4.5753x; 1.3726x over previous
"""Causal multi-head attention block on 8 TRN2 NeuronCores.

Sharding: tensor-parallel over heads (2 heads/core, both batches) for the
QKV projection + attention; an on-device AllToAll re-shards to
sequence-parallel for the output projection (Megatron-style).

v2 structure: QKV projection and attention are fused into one software
pipeline over (batch, q-chunk) — projection of chunk c+1 is emitted
interleaved with attention of chunk c so PE and ACT overlap instead of
running as serial phases.  Init DMAs are batched (one DMA per x chunk via a
4-dim AP, one strided DMA for the VA ones block).  The V projection bias is
folded into the output bias on the host (softmax rows sum to 1), so V needs
no on-device bias add.  Diagonal score tiles truncate the matmul, exp, and
mask to the causal region.

Matmuls run in float32r (full PE rate at N>=256, ~1.5e-4 rel err).  Scores
use zero-padded per-head K^T copies so they run as full K=128 matmuls
(half-height K=64 row-tiled matmuls measured 1.8x slower per-op on HW).

Self-contained: hardcodes all shapes from the problem spec.
"""

import numpy as np
from contextlib import ExitStack

import concourse.bass as bass
import concourse.tile as tile
from concourse import bacc, mybir
from concourse.bass_utils import run_bass_kernel_spmd

F32R = mybir.dt.float32r
F32 = mybir.dt.float32
BF16 = mybir.dt.bfloat16
AF = mybir.ActivationFunctionType

B, T, C, H, HD = 2, 2048, 1024, 16, 64
NCORES = 8
BT = B * T            # 4096 global rows
TQ = 512              # q-chunk width
KT = 128              # k-tile height
NJ = T // TQ          # 4 q-chunks per batch (= per core)
NKK = T // KT         # 16 k-tiles per batch
NCT = C // 128        # 8 contraction tiles for projections
NTC = BT // TQ        # 8 global t-chunks
TSL = BT // NCORES    # 512 rows of final output per core
# chunk-contiguous full x^T: xt[tc0, p, kc, q] — each chunk's load is one
# fully-contiguous DMA (strided HBM reads measured ~8x below spec BW)
XT_SHAPE = [NTC, 128, NCT, TQ]
# pipeline emission style: "hybrid" = first k-tiles of attend(c) emitted
# before the interleaved projection block (keeps ACT fed), "chunk" =
# projections emitted whole before each attend, "phase" = all projections
# then all attends.
INTERLEAVE = "chunk"
V_CONSOL = False      # V quarters share one PSUM tile + 2 big VA copies
ORDER_ROT = False     # rotate batch-1 attends so the tail chunk is short
XPOOL_BUFS = 2
# bf16 input path: x and w_qkv shipped/loaded as bf16 (halves the dominant
# x HBM traffic; QKV projection matmuls run bf16 at the same PE rate).
XT_BF16 = True
# bf16 attention output: y, A2A payload, and w_out in bf16 (halves the
# collective payload and the phase-4 weight/activation traffic).
Y_BF16 = True


def declare_io(nc):
    """DRAM tensors shared by build() and the timing loop builder."""
    xdt = BF16 if XT_BF16 else F32R
    ydt = BF16 if Y_BF16 else F32R
    d = {}
    d["xt"] = nc.dram_tensor("xt", XT_SHAPE, xdt, kind="ExternalInput")
    d["wqkv"] = nc.dram_tensor("wqkv", [128, 3, NCT, 128], xdt,
                               kind="ExternalInput")
    d["bqkv"] = nc.dram_tensor("bqkv", [128, 3], F32, kind="ExternalInput")
    d["wout"] = nc.dram_tensor("wout", [128, NCT, C], ydt,
                               kind="ExternalInput")
    d["bout"] = nc.dram_tensor("bout", [128, C], F32, kind="ExternalInput")
    d["out"] = nc.dram_tensor("out", [TSL, C], F32, kind="ExternalOutput")
    d["vaones"] = nc.dram_tensor("vaones", [128, 2, NKK, 64], F32R,
                                 kind="ExternalInput")
    d["zeros"] = nc.dram_tensor("zeros", [64, T], F32R, kind="ExternalInput")
    return d


def build(with_collective=True):
    nc = bacc.Bacc(None, target_bir_lowering=False)
    d = declare_io(nc)
    ydt = BF16 if Y_BF16 else F32R
    a2a_in = nc.dram_tensor("a2a_in", [NCORES, 128, TQ], ydt)
    if with_collective is None:
        a2a_out = a2a_in
    else:
        a2a_out = nc.dram_tensor("a2a_out", [NCORES, 128, TQ], ydt)
    with tile.TileContext(nc) as tc:
        _emit(nc, tc, d, a2a_in, a2a_out, with_collective)
    nc.compile()
    return nc


def _emit(nc, tc, d, a2a_in, a2a_out, with_collective, trunc=None):
    xt, wqkv, bqkv = d["xt"], d["wqkv"], d["bqkv"]
    wout, bout, out = d["wout"], d["bout"], d["out"]

    with ExitStack() as ctx:
        persist = ctx.enter_context(tc.tile_pool(name="persist", bufs=1))

        # persistent SBUF tensors, indexed by batch b (the core owns the
        # same 2 heads in both batches).
        qts = [persist.tile([128, T], F32R, tag=f"qt{p}", name=f"qt{p}")
               for p in range(2)]
        # zero-padded per-head K^T (head h lives in rows 64*(h%2);
        # the other 64 rows are zero so scores run as full K=128 matmuls)
        kts = [persist.tile([128, T], F32R, tag=f"kt{h}", name=f"kt{h}")
               for h in range(4)]
        xdt = BF16 if XT_BF16 else F32R
        ydt = BF16 if Y_BF16 else F32R
        va = persist.tile([128, 2, NKK, 192], F32R, tag="va")  # [V_e|ones|V_o]
        wsb = persist.tile([128, 3, NCT, 128], xdt, tag="wsb")
        bsb = persist.tile([128, 3], F32, tag="bsb")
        wosb = persist.tile([128, NCT, C], ydt, tag="wo")
        bosb = persist.tile([128, C], F32, tag="bo")

        # per-group weight loads so the Q slice lands first
        for g3 in range(3):
            nc.sync.dma_start(wsb[:, g3], wqkv[:, g3])
        nc.sync.dma_start(bsb[:], bqkv[:])

        def init_rest():
            # deferred so the first x-chunk DMA isn't queued behind these
            # zero the dead half of each per-head K^T (one DMA per head)
            for h in range(4):
                dead = slice(64, 128) if h % 2 == 0 else slice(0, 64)
                nc.sync.dma_start(kts[h][dead, :], d["zeros"][:])
            # ones block of VA in one strided DMA
            nc.sync.dma_start(va[:, :, :, 64:128], d["vaones"][:])

        pipe = ctx.enter_context(ExitStack())
        pools = {}
        pools["x"] = pipe.enter_context(
            tc.tile_pool(name="xtile", bufs=XPOOL_BUFS))
        pools["pp"] = pipe.enter_context(
            tc.tile_pool(name="pp", bufs=2, space="PSUM"))
        pools["s"] = pipe.enter_context(
            tc.tile_pool(name="psc", bufs=2, space="PSUM"))
        pools["o"] = pipe.enter_context(
            tc.tile_pool(name="po", bufs=2, space="PSUM"))
        pools["pt"] = pipe.enter_context(tc.tile_pool(name="ptp", bufs=5))
        pools["yt"] = pipe.enter_context(tc.tile_pool(name="yt", bufs=2))
        pools["rt"] = pipe.enter_context(tc.tile_pool(name="rt", bufs=2))
        pools["oe"] = pipe.enter_context(tc.tile_pool(name="oe", bufs=2))

        def proj_steps(tc0):
            """Generator: emit projection of chunk tc0, yielding between
            units so attention of the previous chunk can interleave."""
            b, jloc = divmod(tc0, NJ)
            chunk = slice(TQ * jloc, TQ * (jloc + 1))
            xtile = pools["x"].tile([128, NCT, TQ], xdt, tag="x",
                                    name=f"x{tc0}")
            nc.sync.dma_start(xtile[:], xt[tc0])
            yield
            for g in range(2):          # 0 = Q^T, 1 = K^T
                ps = pools["pp"].tile([128, TQ], F32, tag="pp",
                                      name=f"pp{tc0}_{g}")
                for kc in range(NCT):
                    nc.tensor.matmul(ps[:], wsb[:, g, kc], xtile[:, kc, :],
                                     start=(kc == 0), stop=(kc == NCT - 1))
                    if kc % 4 == 3:
                        yield
                if g == 0:
                    nc.vector.tensor_scalar_add(qts[b][:, chunk], ps[:],
                                                bsb[:, 0:1])
                else:
                    nc.vector.tensor_scalar_add(
                        kts[2 * b][0:64, chunk], ps[0:64, :], bsb[0:64, 1:2])
                    nc.vector.tensor_scalar_add(
                        kts[2 * b + 1][64:128, chunk], ps[64:128, :],
                        bsb[64:128, 1:2])
                yield
            # V directly in [keys, dims] layout (x^T slice as the stationary
            # operand); V bias is folded into the output bias on the host.
            if V_CONSOL:
                psv = pools["pp"].tile([128, 4, 128], F32, tag="pp",
                                       name=f"ppv{tc0}")
                for q in range(4):
                    for kc in range(NCT):
                        nc.tensor.matmul(psv[:, q],
                                         xtile[:, kc, 128 * q:128 * (q + 1)],
                                         wsb[:, 2, kc],
                                         start=(kc == 0),
                                         stop=(kc == NCT - 1))
                    if q % 2 == 1:
                        yield
                tt4 = slice(jloc * 4, jloc * 4 + 4)  # k-tiles in batch b
                nc.vector.tensor_copy(va[:, b, tt4, 0:64], psv[:, :, 0:64])
                nc.vector.tensor_copy(va[:, b, tt4, 128:192],
                                      psv[:, :, 64:128])
                yield
            else:
                for q in range(4):
                    tt = jloc * 4 + q   # k-tile index in batch b
                    psv = pools["pp"].tile([128, TQ], F32, tag="pp",
                                           name=f"ppv{tc0}_{q}")
                    for kc in range(NCT):
                        nc.tensor.matmul(psv[:, 0:128],
                                         xtile[:, kc, 128 * q:128 * (q + 1)],
                                         wsb[:, 2, kc],
                                         start=(kc == 0),
                                         stop=(kc == NCT - 1))
                    nc.vector.tensor_copy(va[:, b, tt, 0:64], psv[:, 0:64])
                    nc.vector.tensor_copy(va[:, b, tt, 128:192],
                                          psv[:, 64:128])
                    yield

        def attend_steps(p, j):
            """Generator: emit attention for q-chunk (p, j), yielding after
            each k-tile."""
            nkk = 4 * (j + 1)
            po = [pools["o"].tile([128, TQ], F32, tag="po",
                                  name=f"po{p}_{j}_{h}") for h in range(2)]
            for kk in range(nkk):
                o = max(kk - 4 * j, 0)  # suffix offset (diagonal tiles)
                lo = KT * o
                ps_s = pools["s"].tile([128, 2, TQ], F32, tag="s",
                                       name=f"s{p}_{j}_{kk}")
                for h2 in range(2):
                    nc.tensor.matmul(
                        ps_s[:, h2, lo:],
                        kts[2 * p + h2][:, KT * kk:KT * (kk + 1)],
                        qts[p][:, TQ * j + lo:TQ * (j + 1)],
                        start=True, stop=True)
                pt = pools["pt"].tile([128, 2, TQ], F32R, tag="pt",
                                      name=f"p{p}_{j}_{kk}")
                nc.scalar.activation(pt[:, :, lo:], ps_s[:, :, lo:], AF.Exp)
                if kk >= 4 * j:
                    for h2 in range(2):
                        # aligned triangle: keep qf' >= r
                        nc.gpsimd.affine_select(
                            out=pt[:, h2, lo:],
                            in_=pt[:, h2, lo:],
                            compare_op=mybir.AluOpType.is_ge,
                            fill=0.0, base=0,
                            pattern=[[1, TQ - lo]],
                            channel_multiplier=-1)
                for h2 in range(2):
                    vs = slice(0, 128) if h2 == 0 else slice(64, 192)
                    nc.tensor.matmul(
                        po[h2][:, lo:], va[:, p, kk, vs], pt[:, h2, lo:],
                        start=(kk == 0), stop=(kk == nkk - 1))
                yield
            # normalize: h0 sums in rows 64:128, h1 sums in rows 0:64
            # (copy psum->sbuf fast so the accumulator banks free early)
            oes = [pools["oe"].tile([128, TQ], F32, tag="oe",
                                    name=f"oe{p}_{j}_{h}") for h in range(2)]
            nc.vector.tensor_copy(oes[0][:], po[0][:])
            nc.vector.tensor_copy(oes[1][:], po[1][:])
            yt = pools["yt"].tile([128, TQ], ydt, tag="yt", name=f"y{p}_{j}")
            rt = pools["rt"].tile([128, TQ], F32, tag="rt", name=f"r{p}_{j}")
            nc.vector.reciprocal(rt[0:64, :], oes[0][64:128, :])
            nc.vector.tensor_mul(yt[0:64, :], oes[0][0:64, :], rt[0:64, :])
            nc.vector.reciprocal(rt[64:128, :], oes[1][0:64, :])
            nc.vector.tensor_mul(yt[64:128, :], oes[1][64:128, :],
                                 rt[64:128, :])
            nc.sync.dma_start(a2a_in[p * NJ + j, :, :], yt[:])

        # ---- fused pipeline: proj(c+1) interleaved with attend(c) ----
        def drain(g):
            if g is not None:
                for _ in g:
                    pass

        # batch 1 optionally rotated so the last attend chunk is a short one
        # (4 k-tiles), shrinking the serial tail before the A2A.
        if ORDER_ROT:
            order = [(0, 0), (0, 1), (0, 2), (0, 3),
                     (1, 1), (1, 2), (1, 3), (1, 0)]
        else:
            order = [(p, j) for p in range(2) for j in range(NJ)]
        # one projection per attend step (lookahead 2, chunks 0,1 up front)
        proj_plan = {i: [i + 2] for i in range(NTC - 2)}

        def start_proj(idx):
            g = proj_steps(idx)
            next(g)          # emits the x-chunk DMA
            if idx == 0:
                init_rest()
            return g

        if trunc == "proj" or INTERLEAVE == "phase":
            for idx in range(NTC):
                drain(start_proj(idx))
            if trunc != "proj":
                for idx, (p, j) in enumerate(order):
                    if idx == 4:
                        nc.sync.dma_start(wosb[:], wout[:])
                        nc.sync.dma_start(bosb[:], bout[:])
                    drain(attend_steps(p, j))
        else:
            drain(start_proj(0))
            drain(start_proj(1))
            for idx, (p, j) in enumerate(order):
                if idx == 4:
                    nc.sync.dma_start(wosb[:], wout[:])
                    nc.sync.dma_start(bosb[:], bout[:])
                pgs = [start_proj(k) for k in proj_plan.get(idx, [])]
                ag = attend_steps(p, j)
                if INTERLEAVE == "hybrid":
                    # prime ACT with the first k-tiles before the proj block
                    for _ in range(2):
                        next(ag, None)
                for pg in pgs:
                    drain(pg)
                drain(ag)

        pipe.close()

        if trunc in ("proj", "attn"):
            with tc.tile_pool(name="dumo", bufs=1) as dpool:
                dm = dpool.tile([128, TQ], F32, tag="d")
                nc.vector.tensor_copy(dm[:], qts[0][0:128, 0:TQ].bitcast(F32))
                nc.sync.dma_start(out[0:128, 0:TQ], dm[:])
            return

        # ---- all-to-all (head-sharded -> t-sharded) ----
        if with_collective is True:
            nc.gpsimd.collective_compute(
                "AllToAll", mybir.AluOpType.bypass,
                replica_groups=[list(range(NCORES))],
                ins=[a2a_in[:]], outs=[a2a_out[:]])
        elif with_collective is False:
            nc.sync.dma_start(a2a_out[:], a2a_in[:])
        # else (None): timing mode — caller aliases a2a_out to a2a_in

        # ---- output projection (rows TSL per core) ----
        with (
            tc.tile_pool(name="yts", bufs=1) as ytspool,
            tc.tile_pool(name="pout", bufs=4, space="PSUM") as poutp,
            tc.tile_pool(name="osb", bufs=4) as osbpool,
        ):
            yts = ytspool.tile([128, NCT, TQ], ydt, tag="yts")
            for cc in range(NCT):
                nc.sync.dma_start(yts[:, cc, :], a2a_out[cc, :, :])

            for tt in range(TSL // 128):
                pos = [poutp.tile([128, TQ], F32, tag="pout",
                                  name=f"pos{tt}_{h}") for h in range(2)]
                for cc in range(NCT):
                    for n in range(2):
                        nc.tensor.matmul(
                            pos[n][:], yts[:, cc, 128 * tt:128 * (tt + 1)],
                            wosb[:, cc, TQ * n:TQ * (n + 1)],
                            start=(cc == 0), stop=(cc == NCT - 1))
                for n in range(2):
                    osb = osbpool.tile([128, TQ], F32, tag="osb")
                    nc.vector.tensor_add(osb[:], pos[n][:],
                                         bosb[:, TQ * n:TQ * (n + 1)])
                    nc.sync.dma_start(
                        out[128 * tt:128 * (tt + 1), TQ * n:TQ * (n + 1)],
                        osb[:])


def make_core_inputs(x, w_qkv, b_qkv, w_out, b_out):
    """Host-side shard/transform. Returns list of per-core input dicts."""
    x = np.asarray(x, np.float32)
    w_qkv = np.asarray(w_qkv, np.float32)
    b_qkv = np.asarray(b_qkv, np.float32)
    w_out = np.asarray(w_out, np.float32)
    b_out = np.asarray(b_out, np.float32)

    import ml_dtypes
    xdt = ml_dtypes.bfloat16 if XT_BF16 else np.float32
    ydt = ml_dtypes.bfloat16 if Y_BF16 else np.float32

    # softmax rows sum to 1, so the V bias contributes (b_v @ w_out) to
    # every output row — fold it into the output bias.
    b_eff = b_out + b_qkv[2 * C:] @ w_out.astype(ydt).astype(np.float32)
    bout_rep = np.ascontiguousarray(
        np.broadcast_to(b_eff.astype(np.float32), (128, C)))
    # chunk-contiguous x^T: xt[tc0, p, kc, q] = x_flat[TQ*tc0+q, 128*kc+p]
    xt = np.ascontiguousarray(
        x.reshape(NTC, TQ, NCT, 128).transpose(0, 3, 2, 1)).astype(xdt)
    # weight layouts matching the on-device SBUF tiles (contiguous DMAs)
    wout2 = np.ascontiguousarray(
        w_out.reshape(NCT, 128, C).transpose(1, 0, 2)).astype(ydt)
    vaones = np.ones((128, 2, NKK, 64), np.float32)
    zeros = np.zeros((64, T), np.float32)
    in_maps = []
    for c in range(NCORES):
        s = slice(128 * c, 128 * (c + 1))
        wq = w_qkv[:, :C][:, s] * 0.125
        wk = w_qkv[:, C:2 * C][:, s]
        wv = w_qkv[:, 2 * C:][:, s]
        wc = np.concatenate([wq, wk, wv], axis=1)
        wc2 = np.ascontiguousarray(
            wc.reshape(NCT, 128, 3, 128).transpose(1, 2, 0, 3)).astype(xdt)
        bc3 = np.ascontiguousarray(
            np.stack([b_qkv[:C][s] * 0.125, b_qkv[C:2 * C][s],
                      np.zeros(128, np.float32)], axis=1))
        in_maps.append({
            "xt": xt, "wqkv": wc2, "bqkv": bc3,
            "wout": wout2, "bout": bout_rep,
            "vaones": vaones, "zeros": zeros,
        })
    return in_maps


_NC_CACHE = {}


def kernel(x, w_qkv, b_qkv, w_out, b_out):
    in_maps = make_core_inputs(x, w_qkv, b_qkv, w_out, b_out)
    if "nc" not in _NC_CACHE:
        _NC_CACHE["nc"] = build()
    nc = _NC_CACHE["nc"]
    res = run_bass_kernel_spmd(nc, in_maps, core_ids=list(range(NCORES)))
    full = np.concatenate([res.results[c]["out"] for c in range(NCORES)],
                          axis=0)
    return full.reshape(B, T, C)


# revision 48
# speedup vs baseline: 4.6552x; 1.0175x over previous
"""Causal multi-head attention block on 8 TRN2 NeuronCores.

Sharding: tensor-parallel over heads (2 heads/core, both batches) for the
QKV projection + attention; an on-device AllToAll re-shards to
sequence-parallel for the output projection (Megatron-style).

v2 structure: QKV projection and attention are fused into one software
pipeline over (batch, q-chunk) — projection of chunk c+1 is emitted
interleaved with attention of chunk c so PE and ACT overlap instead of
running as serial phases.  Init DMAs are batched (one DMA per x chunk via a
4-dim AP, one strided DMA for the VA ones block).  The V projection bias is
folded into the output bias on the host (softmax rows sum to 1), so V needs
no on-device bias add.  Diagonal score tiles truncate the matmul, exp, and
mask to the causal region.

Matmuls run in float32r (full PE rate at N>=256, ~1.5e-4 rel err).  Scores
use zero-padded per-head K^T copies so they run as full K=128 matmuls
(half-height K=64 row-tiled matmuls measured 1.8x slower per-op on HW).

Self-contained: hardcodes all shapes from the problem spec.
"""

import numpy as np
from contextlib import ExitStack

import concourse.bass as bass
import concourse.tile as tile
from concourse import bacc, mybir
from concourse.bass_utils import run_bass_kernel_spmd

F32R = mybir.dt.float32r
F32 = mybir.dt.float32
BF16 = mybir.dt.bfloat16
AF = mybir.ActivationFunctionType

B, T, C, H, HD = 2, 2048, 1024, 16, 64
NCORES = 8
BT = B * T            # 4096 global rows
TQ = 512              # q-chunk width
KT = 128              # k-tile height
NJ = T // TQ          # 4 q-chunks per batch (= per core)
NKK = T // KT         # 16 k-tiles per batch
NCT = C // 128        # 8 contraction tiles for projections
NTC = BT // TQ        # 8 global t-chunks
TSL = BT // NCORES    # 512 rows of final output per core
# chunk-contiguous full x^T: xt[tc0, p, kc, q] — each chunk's load is one
# fully-contiguous DMA (strided HBM reads measured ~8x below spec BW)
XT_SHAPE = [NTC, 128, NCT, TQ]
# pipeline emission style: "hybrid" = first k-tiles of attend(c) emitted
# before the interleaved projection block (keeps ACT fed), "chunk" =
# projections emitted whole before each attend, "phase" = all projections
# then all attends.
INTERLEAVE = "chunk"
V_CONSOL = False      # V quarters share one PSUM tile + 2 big VA copies
ORDER_ROT = False     # rotate batch-1 attends so the tail chunk is short
XPOOL_BUFS = 2
PLAN_EARLY = False    # attend(0,0) starts right after proj(0)
# bf16 input path: x and w_qkv shipped/loaded as bf16 (halves the dominant
# x HBM traffic; QKV projection matmuls run bf16 at the same PE rate).
XT_BF16 = True
# bf16 attention output: y, A2A payload, and w_out in bf16 (halves the
# collective payload and the phase-4 weight/activation traffic).
Y_BF16 = True


def declare_io(nc):
    """DRAM tensors shared by build() and the timing loop builder."""
    xdt = BF16 if XT_BF16 else F32R
    ydt = BF16 if Y_BF16 else F32R
    d = {}
    d["xt"] = nc.dram_tensor("xt", XT_SHAPE, xdt, kind="ExternalInput")
    d["wqkv"] = nc.dram_tensor("wqkv", [128, 3, NCT, 128], xdt,
                               kind="ExternalInput")
    d["bqkv"] = nc.dram_tensor("bqkv", [128, 3], F32, kind="ExternalInput")
    d["wout"] = nc.dram_tensor("wout", [128, NCT, C], ydt,
                               kind="ExternalInput")
    d["bout"] = nc.dram_tensor("bout", [128, C], F32, kind="ExternalInput")
    d["out"] = nc.dram_tensor("out", [TSL, C], F32, kind="ExternalOutput")
    d["vaones"] = nc.dram_tensor("vaones", [128, 2, NKK, 64], F32R,
                                 kind="ExternalInput")
    d["zeros"] = nc.dram_tensor("zeros", [64, T], F32R, kind="ExternalInput")
    return d


def build(with_collective=True):
    nc = bacc.Bacc(None, target_bir_lowering=False)
    d = declare_io(nc)
    ydt = BF16 if Y_BF16 else F32R
    a2a_in = nc.dram_tensor("a2a_in", [NCORES, 128, TQ], ydt)
    if with_collective is None:
        a2a_out = a2a_in
    else:
        a2a_out = nc.dram_tensor("a2a_out", [NCORES, 128, TQ], ydt)
    with tile.TileContext(nc) as tc:
        _emit(nc, tc, d, a2a_in, a2a_out, with_collective)
    nc.compile()
    return nc


def _emit(nc, tc, d, a2a_in, a2a_out, with_collective, trunc=None):
    xt, wqkv, bqkv = d["xt"], d["wqkv"], d["bqkv"]
    wout, bout, out = d["wout"], d["bout"], d["out"]

    with ExitStack() as ctx:
        persist = ctx.enter_context(tc.tile_pool(name="persist", bufs=1))

        # persistent SBUF tensors, indexed by batch b (the core owns the
        # same 2 heads in both batches).
        qts = [persist.tile([128, T], F32R, tag=f"qt{p}", name=f"qt{p}")
               for p in range(2)]
        # zero-padded per-head K^T (head h lives in rows 64*(h%2);
        # the other 64 rows are zero so scores run as full K=128 matmuls)
        kts = [persist.tile([128, T], F32R, tag=f"kt{h}", name=f"kt{h}")
               for h in range(4)]
        xdt = BF16 if XT_BF16 else F32R
        ydt = BF16 if Y_BF16 else F32R
        va = persist.tile([128, 2, NKK, 192], F32R, tag="va")  # [V_e|ones|V_o]
        wsb = persist.tile([128, 3, NCT, 128], xdt, tag="wsb")
        bsb = persist.tile([128, 3], F32, tag="bsb")
        wosb = persist.tile([128, NCT, C], ydt, tag="wo")
        bosb = persist.tile([128, C], F32, tag="bo")

        # per-group weight loads so the Q slice lands first
        for g3 in range(3):
            nc.sync.dma_start(wsb[:, g3], wqkv[:, g3])
        nc.sync.dma_start(bsb[:], bqkv[:])

        def init_rest():
            # deferred so the first x-chunk DMA isn't queued behind these
            # zero the dead half of each per-head K^T (one DMA per head)
            for h in range(4):
                dead = slice(64, 128) if h % 2 == 0 else slice(0, 64)
                nc.sync.dma_start(kts[h][dead, :], d["zeros"][:])
            # ones block of VA in one strided DMA
            nc.sync.dma_start(va[:, :, :, 64:128], d["vaones"][:])

        pipe = ctx.enter_context(ExitStack())
        pools = {}
        pools["x"] = pipe.enter_context(
            tc.tile_pool(name="xtile", bufs=XPOOL_BUFS))
        pools["pp"] = pipe.enter_context(
            tc.tile_pool(name="pp", bufs=2, space="PSUM"))
        pools["s"] = pipe.enter_context(
            tc.tile_pool(name="psc", bufs=2, space="PSUM"))
        pools["o"] = pipe.enter_context(
            tc.tile_pool(name="po", bufs=2, space="PSUM"))
        pools["pt"] = pipe.enter_context(tc.tile_pool(name="ptp", bufs=5))
        pools["yt"] = pipe.enter_context(tc.tile_pool(name="yt", bufs=2))
        pools["rt"] = pipe.enter_context(tc.tile_pool(name="rt", bufs=2))
        pools["oe"] = pipe.enter_context(tc.tile_pool(name="oe", bufs=2))

        def proj_steps(tc0):
            """Generator: emit projection of chunk tc0, yielding between
            units so attention of the previous chunk can interleave."""
            b, jloc = divmod(tc0, NJ)
            chunk = slice(TQ * jloc, TQ * (jloc + 1))
            xtile = pools["x"].tile([128, NCT, TQ], xdt, tag="x",
                                    name=f"x{tc0}")
            nc.sync.dma_start(xtile[:], xt[tc0])
            yield
            for g in range(2):          # 0 = Q^T, 1 = K^T
                ps = pools["pp"].tile([128, TQ], F32, tag="pp",
                                      name=f"pp{tc0}_{g}")
                for kc in range(NCT):
                    nc.tensor.matmul(ps[:], wsb[:, g, kc], xtile[:, kc, :],
                                     start=(kc == 0), stop=(kc == NCT - 1))
                    if kc % 2 == 1:
                        yield
                if g == 0:
                    nc.vector.tensor_scalar_add(qts[b][:, chunk], ps[:],
                                                bsb[:, 0:1])
                else:
                    nc.vector.tensor_scalar_add(
                        kts[2 * b][0:64, chunk], ps[0:64, :], bsb[0:64, 1:2])
                    nc.vector.tensor_scalar_add(
                        kts[2 * b + 1][64:128, chunk], ps[64:128, :],
                        bsb[64:128, 1:2])
                yield
            # V directly in [keys, dims] layout (x^T slice as the stationary
            # operand); V bias is folded into the output bias on the host.
            if V_CONSOL:
                psv = pools["pp"].tile([128, 4, 128], F32, tag="pp",
                                       name=f"ppv{tc0}")
                for q in range(4):
                    for kc in range(NCT):
                        nc.tensor.matmul(psv[:, q],
                                         xtile[:, kc, 128 * q:128 * (q + 1)],
                                         wsb[:, 2, kc],
                                         start=(kc == 0),
                                         stop=(kc == NCT - 1))
                    if q % 2 == 1:
                        yield
                tt4 = slice(jloc * 4, jloc * 4 + 4)  # k-tiles in batch b
                nc.vector.tensor_copy(va[:, b, tt4, 0:64], psv[:, :, 0:64])
                nc.vector.tensor_copy(va[:, b, tt4, 128:192],
                                      psv[:, :, 64:128])
                yield
            else:
                for q in range(4):
                    tt = jloc * 4 + q   # k-tile index in batch b
                    psv = pools["pp"].tile([128, TQ], F32, tag="pp",
                                           name=f"ppv{tc0}_{q}")
                    for kc in range(NCT):
                        nc.tensor.matmul(psv[:, 0:128],
                                         xtile[:, kc, 128 * q:128 * (q + 1)],
                                         wsb[:, 2, kc],
                                         start=(kc == 0),
                                         stop=(kc == NCT - 1))
                    nc.vector.tensor_copy(va[:, b, tt, 0:64], psv[:, 0:64])
                    nc.vector.tensor_copy(va[:, b, tt, 128:192],
                                          psv[:, 64:128])
                    yield

        def attend_steps(p, j):
            """Generator: emit attention for q-chunk (p, j), yielding after
            each k-tile."""
            nkk = 4 * (j + 1)
            po = [pools["o"].tile([128, TQ], F32, tag="po",
                                  name=f"po{p}_{j}_{h}") for h in range(2)]
            for kk in range(nkk):
                o = max(kk - 4 * j, 0)  # suffix offset (diagonal tiles)
                lo = KT * o
                ps_s = pools["s"].tile([128, 2, TQ], F32, tag="s",
                                       name=f"s{p}_{j}_{kk}")
                for h2 in range(2):
                    nc.tensor.matmul(
                        ps_s[:, h2, lo:],
                        kts[2 * p + h2][:, KT * kk:KT * (kk + 1)],
                        qts[p][:, TQ * j + lo:TQ * (j + 1)],
                        start=True, stop=True)
                pt = pools["pt"].tile([128, 2, TQ], F32R, tag="pt",
                                      name=f"p{p}_{j}_{kk}")
                nc.scalar.activation(pt[:, :, lo:], ps_s[:, :, lo:], AF.Exp)
                if kk >= 4 * j:
                    for h2 in range(2):
                        # aligned triangle: keep qf' >= r
                        nc.gpsimd.affine_select(
                            out=pt[:, h2, lo:],
                            in_=pt[:, h2, lo:],
                            compare_op=mybir.AluOpType.is_ge,
                            fill=0.0, base=0,
                            pattern=[[1, TQ - lo]],
                            channel_multiplier=-1)
                # yield here so filler PE work can cover the exp latency
                # between this k-tile's score and AV matmuls
                yield
                for h2 in range(2):
                    vs = slice(0, 128) if h2 == 0 else slice(64, 192)
                    nc.tensor.matmul(
                        po[h2][:, lo:], va[:, p, kk, vs], pt[:, h2, lo:],
                        start=(kk == 0), stop=(kk == nkk - 1))
                yield
            # normalize: h0 sums in rows 64:128, h1 sums in rows 0:64
            # (copy psum->sbuf fast so the accumulator banks free early)
            oes = [pools["oe"].tile([128, TQ], F32, tag="oe",
                                    name=f"oe{p}_{j}_{h}") for h in range(2)]
            nc.vector.tensor_copy(oes[0][:], po[0][:])
            nc.vector.tensor_copy(oes[1][:], po[1][:])
            yt = pools["yt"].tile([128, TQ], ydt, tag="yt", name=f"y{p}_{j}")
            rt = pools["rt"].tile([128, TQ], F32, tag="rt", name=f"r{p}_{j}")
            nc.vector.reciprocal(rt[0:64, :], oes[0][64:128, :])
            nc.vector.tensor_mul(yt[0:64, :], oes[0][0:64, :], rt[0:64, :])
            nc.vector.reciprocal(rt[64:128, :], oes[1][0:64, :])
            nc.vector.tensor_mul(yt[64:128, :], oes[1][64:128, :],
                                 rt[64:128, :])
            nc.sync.dma_start(a2a_in[p * NJ + j, :, :], yt[:])

        # ---- fused pipeline: proj(c+1) interleaved with attend(c) ----
        def drain(g):
            if g is not None:
                for _ in g:
                    pass

        # batch 1 optionally rotated so the last attend chunk is a short one
        # (4 k-tiles), shrinking the serial tail before the A2A.
        if ORDER_ROT:
            order = [(0, 0), (0, 1), (0, 2), (0, 3),
                     (1, 1), (1, 2), (1, 3), (1, 0)]
        else:
            order = [(p, j) for p in range(2) for j in range(NJ)]
        # projection emission plan: chunk 0 up front, attend(0,0) starts
        # immediately after it, then one projection block per attend step
        if PLAN_EARLY:
            proj_plan = {1: [1, 2], 2: [3], 3: [4], 4: [5], 5: [6], 6: [7]}
            upfront = [0]
        else:
            proj_plan = {i: [i + 2] for i in range(NTC - 2)}
            upfront = [0, 1]

        def start_proj(idx):
            g = proj_steps(idx)
            next(g)          # emits the x-chunk DMA
            if idx == 0:
                init_rest()
            return g

        if trunc == "proj" or INTERLEAVE == "phase":
            for idx in range(NTC):
                drain(start_proj(idx))
            if trunc != "proj":
                for idx, (p, j) in enumerate(order):
                    if idx == 4:
                        nc.sync.dma_start(wosb[:], wout[:])
                        nc.sync.dma_start(bosb[:], bout[:])
                    drain(attend_steps(p, j))
        else:
            for k in upfront:
                drain(start_proj(k))
            for idx, (p, j) in enumerate(order):
                if idx == 4:
                    nc.sync.dma_start(wosb[:], wout[:])
                    nc.sync.dma_start(bosb[:], bout[:])
                pgs = [start_proj(k) for k in proj_plan.get(idx, [])]
                ag = attend_steps(p, j)
                if INTERLEAVE == "fill" and pgs:
                    # pace projection units into the attend yield points --
                    # one sits between each k-tile's score and AV matmuls,
                    # covering the exp latency with PE work
                    from itertools import chain
                    pchain = chain(*pgs)
                    ny = 8 * (j + 1)
                    np_est = 15 * len(pgs)
                    acc = 0
                    for i, _ in enumerate(ag):
                        want = ((i + 1) * np_est) // ny
                        while pchain is not None and acc < want:
                            if next(pchain, "end") == "end":
                                pchain = None
                                break
                            acc += 1
                    drain(pchain)
                    continue
                if INTERLEAVE == "hybrid":
                    # prime ACT with the first k-tiles before the proj block
                    for _ in range(2):
                        next(ag, None)
                for pg in pgs:
                    drain(pg)
                drain(ag)

        pipe.close()

        if trunc in ("proj", "attn"):
            with tc.tile_pool(name="dumo", bufs=1) as dpool:
                dm = dpool.tile([128, TQ], F32, tag="d")
                nc.vector.tensor_copy(dm[:], qts[0][0:128, 0:TQ].bitcast(F32))
                nc.sync.dma_start(out[0:128, 0:TQ], dm[:])
            return

        # ---- all-to-all (head-sharded -> t-sharded) ----
        if with_collective is True:
            nc.gpsimd.collective_compute(
                "AllToAll", mybir.AluOpType.bypass,
                replica_groups=[list(range(NCORES))],
                ins=[a2a_in[:]], outs=[a2a_out[:]])
        elif with_collective is False:
            nc.sync.dma_start(a2a_out[:], a2a_in[:])
        # else (None): timing mode — caller aliases a2a_out to a2a_in

        # ---- output projection (rows TSL per core) ----
        with (
            tc.tile_pool(name="yts", bufs=1) as ytspool,
            tc.tile_pool(name="pout", bufs=4, space="PSUM") as poutp,
            tc.tile_pool(name="osb", bufs=4) as osbpool,
        ):
            yts = ytspool.tile([128, NCT, TQ], ydt, tag="yts")
            for cc in range(NCT):
                nc.sync.dma_start(yts[:, cc, :], a2a_out[cc, :, :])

            for tt in range(TSL // 128):
                pos = [poutp.tile([128, TQ], F32, tag="pout",
                                  name=f"pos{tt}_{h}") for h in range(2)]
                for cc in range(NCT):
                    for n in range(2):
                        nc.tensor.matmul(
                            pos[n][:], yts[:, cc, 128 * tt:128 * (tt + 1)],
                            wosb[:, cc, TQ * n:TQ * (n + 1)],
                            start=(cc == 0), stop=(cc == NCT - 1))
                for n in range(2):
                    osb = osbpool.tile([128, TQ], F32, tag="osb")
                    nc.vector.tensor_add(osb[:], pos[n][:],
                                         bosb[:, TQ * n:TQ * (n + 1)])
                    nc.sync.dma_start(
                        out[128 * tt:128 * (tt + 1), TQ * n:TQ * (n + 1)],
                        osb[:])


def make_core_inputs(x, w_qkv, b_qkv, w_out, b_out):
    """Host-side shard/transform. Returns list of per-core input dicts."""
    x = np.asarray(x, np.float32)
    w_qkv = np.asarray(w_qkv, np.float32)
    b_qkv = np.asarray(b_qkv, np.float32)
    w_out = np.asarray(w_out, np.float32)
    b_out = np.asarray(b_out, np.float32)

    import ml_dtypes
    xdt = ml_dtypes.bfloat16 if XT_BF16 else np.float32
    ydt = ml_dtypes.bfloat16 if Y_BF16 else np.float32

    # softmax rows sum to 1, so the V bias contributes (b_v @ w_out) to
    # every output row — fold it into the output bias.
    b_eff = b_out + b_qkv[2 * C:] @ w_out.astype(ydt).astype(np.float32)
    bout_rep = np.ascontiguousarray(
        np.broadcast_to(b_eff.astype(np.float32), (128, C)))
    # chunk-contiguous x^T: xt[tc0, p, kc, q] = x_flat[TQ*tc0+q, 128*kc+p]
    xt = np.ascontiguousarray(
        x.reshape(NTC, TQ, NCT, 128).transpose(0, 3, 2, 1)).astype(xdt)
    # weight layouts matching the on-device SBUF tiles (contiguous DMAs)
    wout2 = np.ascontiguousarray(
        w_out.reshape(NCT, 128, C).transpose(1, 0, 2)).astype(ydt)
    vaones = np.ones((128, 2, NKK, 64), np.float32)
    zeros = np.zeros((64, T), np.float32)
    in_maps = []
    for c in range(NCORES):
        s = slice(128 * c, 128 * (c + 1))
        wq = w_qkv[:, :C][:, s] * 0.125
        wk = w_qkv[:, C:2 * C][:, s]
        wv = w_qkv[:, 2 * C:][:, s]
        wc = np.concatenate([wq, wk, wv], axis=1)
        wc2 = np.ascontiguousarray(
            wc.reshape(NCT, 128, 3, 128).transpose(1, 2, 0, 3)).astype(xdt)
        bc3 = np.ascontiguousarray(
            np.stack([b_qkv[:C][s] * 0.125, b_qkv[C:2 * C][s],
                      np.zeros(128, np.float32)], axis=1))
        in_maps.append({
            "xt": xt, "wqkv": wc2, "bqkv": bc3,
            "wout": wout2, "bout": bout_rep,
            "vaones": vaones, "zeros": zeros,
        })
    return in_maps


_NC_CACHE = {}


def kernel(x, w_qkv, b_qkv, w_out, b_out):
    in_maps = make_core_inputs(x, w_qkv, b_qkv, w_out, b_out)
    if "nc" not in _NC_CACHE:
        _NC_CACHE["nc"] = build()
    nc = _NC_CACHE["nc"]
    res = run_bass_kernel_spmd(nc, in_maps, core_ids=list(range(NCORES)))
    full = np.concatenate([res.results[c]["out"] for c in range(NCORES)],
                          axis=0)
    return full.reshape(B, T, C)


# revision 54
# speedup vs baseline: 4.6989x; 1.0094x over previous
"""Causal multi-head attention block on 8 TRN2 NeuronCores.

Sharding: tensor-parallel over heads (2 heads/core, both batches) for the
QKV projection + attention; an on-device AllToAll re-shards to
sequence-parallel for the output projection (Megatron-style).

v2 structure: QKV projection and attention are fused into one software
pipeline over (batch, q-chunk) — projection of chunk c+1 is emitted
interleaved with attention of chunk c so PE and ACT overlap instead of
running as serial phases.  Init DMAs are batched (one DMA per x chunk via a
4-dim AP, one strided DMA for the VA ones block).  The V projection bias is
folded into the output bias on the host (softmax rows sum to 1), so V needs
no on-device bias add.  Diagonal score tiles truncate the matmul, exp, and
mask to the causal region.

Matmuls run in float32r (full PE rate at N>=256, ~1.5e-4 rel err).  Scores
use zero-padded per-head K^T copies so they run as full K=128 matmuls
(half-height K=64 row-tiled matmuls measured 1.8x slower per-op on HW).

Self-contained: hardcodes all shapes from the problem spec.
"""

import numpy as np
from contextlib import ExitStack

import concourse.bass as bass
import concourse.tile as tile
from concourse import bacc, mybir
from concourse.bass_utils import run_bass_kernel_spmd

F32R = mybir.dt.float32r
F32 = mybir.dt.float32
BF16 = mybir.dt.bfloat16
AF = mybir.ActivationFunctionType

B, T, C, H, HD = 2, 2048, 1024, 16, 64
NCORES = 8
BT = B * T            # 4096 global rows
TQ = 512              # q-chunk width
KT = 128              # k-tile height
NJ = T // TQ          # 4 q-chunks per batch (= per core)
NKK = T // KT         # 16 k-tiles per batch
NCT = C // 128        # 8 contraction tiles for projections
NTC = BT // TQ        # 8 global t-chunks
TSL = BT // NCORES    # 512 rows of final output per core
# chunk-contiguous full x^T: xt[tc0, p, kc, q] — each chunk's load is one
# fully-contiguous DMA (strided HBM reads measured ~8x below spec BW)
XT_SHAPE = [NTC, 128, NCT, TQ]
# pipeline emission style: "hybrid" = first k-tiles of attend(c) emitted
# before the interleaved projection block (keeps ACT fed), "chunk" =
# projections emitted whole before each attend, "phase" = all projections
# then all attends.
INTERLEAVE = "chunk"
V_CONSOL = False      # V quarters share one PSUM tile + 2 big VA copies
ORDER_ROT = False     # rotate batch-1 attends so the tail chunk is short
XPOOL_BUFS = 2
PLAN_EARLY = False    # attend(0,0) starts right after proj(0)
SEL1 = False          # one affine_select covers both heads' mask regions
PSUM_SHARE = False    # projection PSUM shares the score pool (3 slots)
# bf16 input path: x and w_qkv shipped/loaded as bf16 (halves the dominant
# x HBM traffic; QKV projection matmuls run bf16 at the same PE rate).
XT_BF16 = True
# bf16 attention output: y, A2A payload, and w_out in bf16 (halves the
# collective payload and the phase-4 weight/activation traffic).
Y_BF16 = True


def declare_io(nc):
    """DRAM tensors shared by build() and the timing loop builder."""
    xdt = BF16 if XT_BF16 else F32R
    ydt = BF16 if Y_BF16 else F32R
    d = {}
    d["xt"] = nc.dram_tensor("xt", XT_SHAPE, xdt, kind="ExternalInput")
    d["wqkv"] = nc.dram_tensor("wqkv", [128, 3, NCT, 128], xdt,
                               kind="ExternalInput")
    d["bqkv"] = nc.dram_tensor("bqkv", [128, 3], F32, kind="ExternalInput")
    d["wout"] = nc.dram_tensor("wout", [128, NCT, C], ydt,
                               kind="ExternalInput")
    d["bout"] = nc.dram_tensor("bout", [128, C], F32, kind="ExternalInput")
    d["out"] = nc.dram_tensor("out", [TSL, C], F32, kind="ExternalOutput")
    d["vaones"] = nc.dram_tensor("vaones", [128, 2, NKK, 64], F32R,
                                 kind="ExternalInput")
    d["zeros"] = nc.dram_tensor("zeros", [64, T], F32R, kind="ExternalInput")
    return d


def build(with_collective=True):
    nc = bacc.Bacc(None, target_bir_lowering=False)
    d = declare_io(nc)
    ydt = BF16 if Y_BF16 else F32R
    a2a_in = nc.dram_tensor("a2a_in", [NCORES, 128, TQ], ydt)
    if with_collective is None:
        a2a_out = a2a_in
    else:
        a2a_out = nc.dram_tensor("a2a_out", [NCORES, 128, TQ], ydt)
    with tile.TileContext(nc) as tc:
        _emit(nc, tc, d, a2a_in, a2a_out, with_collective)
    nc.compile()
    return nc


def _emit(nc, tc, d, a2a_in, a2a_out, with_collective, trunc=None):
    xt, wqkv, bqkv = d["xt"], d["wqkv"], d["bqkv"]
    wout, bout, out = d["wout"], d["bout"], d["out"]

    with ExitStack() as ctx:
        persist = ctx.enter_context(tc.tile_pool(name="persist", bufs=1))

        # persistent SBUF tensors, indexed by batch b (the core owns the
        # same 2 heads in both batches).
        qts = [persist.tile([128, T], F32R, tag=f"qt{p}", name=f"qt{p}")
               for p in range(2)]
        # zero-padded per-head K^T (head h lives in rows 64*(h%2);
        # the other 64 rows are zero so scores run as full K=128 matmuls)
        kts = [persist.tile([128, T], F32R, tag=f"kt{h}", name=f"kt{h}")
               for h in range(4)]
        xdt = BF16 if XT_BF16 else F32R
        ydt = BF16 if Y_BF16 else F32R
        va = persist.tile([128, 2, NKK, 192], F32R, tag="va")  # [V_e|ones|V_o]
        wsb = persist.tile([128, 3, NCT, 128], xdt, tag="wsb")
        bsb = persist.tile([128, 3], F32, tag="bsb")
        wosb = persist.tile([128, NCT, C], ydt, tag="wo")
        bosb = persist.tile([128, C], F32, tag="bo")

        # per-group weight loads so the Q slice lands first
        for g3 in range(3):
            nc.sync.dma_start(wsb[:, g3], wqkv[:, g3])
        nc.sync.dma_start(bsb[:], bqkv[:])

        def init_rest():
            # deferred so the first x-chunk DMA isn't queued behind these
            # zero the dead half of each per-head K^T (one DMA per head)
            for h in range(4):
                dead = slice(64, 128) if h % 2 == 0 else slice(0, 64)
                nc.sync.dma_start(kts[h][dead, :], d["zeros"][:])
            # ones block of VA in one strided DMA
            nc.sync.dma_start(va[:, :, :, 64:128], d["vaones"][:])

        pipe = ctx.enter_context(ExitStack())
        pools = {}
        pools["x"] = pipe.enter_context(
            tc.tile_pool(name="xtile", bufs=XPOOL_BUFS))
        if PSUM_SHARE:
            pools["s"] = pipe.enter_context(
                tc.tile_pool(name="psc", bufs=3, space="PSUM"))
            pools["pp"] = pools["s"]
        else:
            pools["pp"] = pipe.enter_context(
                tc.tile_pool(name="pp", bufs=2, space="PSUM"))
            pools["s"] = pipe.enter_context(
                tc.tile_pool(name="psc", bufs=2, space="PSUM"))
        pools["o"] = pipe.enter_context(
            tc.tile_pool(name="po", bufs=2, space="PSUM"))
        pools["pt"] = pipe.enter_context(tc.tile_pool(name="ptp", bufs=5))
        pools["yt"] = pipe.enter_context(tc.tile_pool(name="yt", bufs=2))
        pools["rt"] = pipe.enter_context(tc.tile_pool(name="rt", bufs=2))
        pools["oe"] = pipe.enter_context(tc.tile_pool(name="oe", bufs=2))

        def proj_steps(tc0):
            """Generator: emit projection of chunk tc0, yielding between
            units so attention of the previous chunk can interleave."""
            b, jloc = divmod(tc0, NJ)
            chunk = slice(TQ * jloc, TQ * (jloc + 1))
            xtile = pools["x"].tile([128, NCT, TQ], xdt, tag="x",
                                    name=f"x{tc0}")
            nc.sync.dma_start(xtile[:], xt[tc0])
            yield
            ptag = "s" if PSUM_SHARE else "pp"
            for g in range(2):          # 0 = Q^T, 1 = K^T
                ps = pools["pp"].tile([128, TQ], F32, tag=ptag,
                                      name=f"pp{tc0}_{g}")
                for kc in range(NCT):
                    nc.tensor.matmul(ps[:], wsb[:, g, kc], xtile[:, kc, :],
                                     start=(kc == 0), stop=(kc == NCT - 1))
                    if kc % 2 == 1:
                        yield
                if g == 0:
                    nc.vector.tensor_scalar_add(qts[b][:, chunk], ps[:],
                                                bsb[:, 0:1])
                else:
                    nc.vector.tensor_scalar_add(
                        kts[2 * b][0:64, chunk], ps[0:64, :], bsb[0:64, 1:2])
                    nc.vector.tensor_scalar_add(
                        kts[2 * b + 1][64:128, chunk], ps[64:128, :],
                        bsb[64:128, 1:2])
                yield
            # V directly in [keys, dims] layout (x^T slice as the stationary
            # operand); V bias is folded into the output bias on the host.
            if V_CONSOL:
                psv = pools["pp"].tile([128, 4, 128], F32, tag=ptag,
                                       name=f"ppv{tc0}")
                for q in range(4):
                    for kc in range(NCT):
                        nc.tensor.matmul(psv[:, q],
                                         xtile[:, kc, 128 * q:128 * (q + 1)],
                                         wsb[:, 2, kc],
                                         start=(kc == 0),
                                         stop=(kc == NCT - 1))
                    if q % 2 == 1:
                        yield
                tt4 = slice(jloc * 4, jloc * 4 + 4)  # k-tiles in batch b
                nc.vector.tensor_copy(va[:, b, tt4, 0:64], psv[:, :, 0:64])
                nc.vector.tensor_copy(va[:, b, tt4, 128:192],
                                      psv[:, :, 64:128])
                yield
            else:
                for q in range(4):
                    tt = jloc * 4 + q   # k-tile index in batch b
                    psv = pools["pp"].tile([128, TQ], F32, tag=ptag,
                                           name=f"ppv{tc0}_{q}")
                    for kc in range(NCT):
                        nc.tensor.matmul(psv[:, 0:128],
                                         xtile[:, kc, 128 * q:128 * (q + 1)],
                                         wsb[:, 2, kc],
                                         start=(kc == 0),
                                         stop=(kc == NCT - 1))
                    nc.vector.tensor_copy(va[:, b, tt, 0:64], psv[:, 0:64])
                    nc.vector.tensor_copy(va[:, b, tt, 128:192],
                                          psv[:, 64:128])
                    yield

        def attend_steps(p, j):
            """Generator: emit attention for q-chunk (p, j), yielding after
            each k-tile."""
            nkk = 4 * (j + 1)
            po = [pools["o"].tile([128, TQ], F32, tag="po",
                                  name=f"po{p}_{j}_{h}") for h in range(2)]
            for kk in range(nkk):
                o = max(kk - 4 * j, 0)  # suffix offset (diagonal tiles)
                lo = KT * o
                ps_s = pools["s"].tile([128, 2, TQ], F32, tag="s",
                                       name=f"s{p}_{j}_{kk}")
                for h2 in range(2):
                    nc.tensor.matmul(
                        ps_s[:, h2, lo:],
                        kts[2 * p + h2][:, KT * kk:KT * (kk + 1)],
                        qts[p][:, TQ * j + lo:TQ * (j + 1)],
                        start=True, stop=True)
                pt = pools["pt"].tile([128, 2, TQ], F32R, tag="pt",
                                      name=f"p{p}_{j}_{kk}")
                nc.scalar.activation(pt[:, :, lo:], ps_s[:, :, lo:], AF.Exp)
                if kk >= 4 * j:
                    if SEL1:
                        # aligned triangle for both heads in one op: the
                        # head axis gets affine coefficient 0
                        nc.gpsimd.affine_select(
                            out=pt[:, :, lo:],
                            in_=pt[:, :, lo:],
                            compare_op=mybir.AluOpType.is_ge,
                            fill=0.0, base=0,
                            pattern=[[0, 2], [1, TQ - lo]],
                            channel_multiplier=-1)
                    else:
                        for h2 in range(2):
                            # aligned triangle: keep qf' >= r
                            nc.gpsimd.affine_select(
                                out=pt[:, h2, lo:],
                                in_=pt[:, h2, lo:],
                                compare_op=mybir.AluOpType.is_ge,
                                fill=0.0, base=0,
                                pattern=[[1, TQ - lo]],
                                channel_multiplier=-1)
                # yield here so filler PE work can cover the exp latency
                # between this k-tile's score and AV matmuls
                yield
                for h2 in range(2):
                    vs = slice(0, 128) if h2 == 0 else slice(64, 192)
                    nc.tensor.matmul(
                        po[h2][:, lo:], va[:, p, kk, vs], pt[:, h2, lo:],
                        start=(kk == 0), stop=(kk == nkk - 1))
                yield
            # normalize: h0 sums in rows 64:128, h1 sums in rows 0:64
            # (copy psum->sbuf fast so the accumulator banks free early)
            oes = [pools["oe"].tile([128, TQ], F32, tag="oe",
                                    name=f"oe{p}_{j}_{h}") for h in range(2)]
            nc.vector.tensor_copy(oes[0][:], po[0][:])
            nc.vector.tensor_copy(oes[1][:], po[1][:])
            yt = pools["yt"].tile([128, TQ], ydt, tag="yt", name=f"y{p}_{j}")
            rt = pools["rt"].tile([128, TQ], F32, tag="rt", name=f"r{p}_{j}")
            nc.vector.reciprocal(rt[0:64, :], oes[0][64:128, :])
            nc.vector.tensor_mul(yt[0:64, :], oes[0][0:64, :], rt[0:64, :])
            nc.vector.reciprocal(rt[64:128, :], oes[1][0:64, :])
            nc.vector.tensor_mul(yt[64:128, :], oes[1][64:128, :],
                                 rt[64:128, :])
            nc.sync.dma_start(a2a_in[p * NJ + j, :, :], yt[:])

        # ---- fused pipeline: proj(c+1) interleaved with attend(c) ----
        def drain(g):
            if g is not None:
                for _ in g:
                    pass

        # batch 1 optionally rotated so the last attend chunk is a short one
        # (4 k-tiles), shrinking the serial tail before the A2A.
        if ORDER_ROT:
            order = [(0, 0), (0, 1), (0, 2), (0, 3),
                     (1, 1), (1, 2), (1, 3), (1, 0)]
        else:
            order = [(p, j) for p in range(2) for j in range(NJ)]
        # projection emission plan: chunk 0 up front, attend(0,0) starts
        # immediately after it, then one projection block per attend step
        if PLAN_EARLY:
            proj_plan = {1: [1, 2], 2: [3], 3: [4], 4: [5], 5: [6], 6: [7]}
            upfront = [0]
        else:
            proj_plan = {i: [i + 2] for i in range(NTC - 2)}
            upfront = [0, 1]

        def start_proj(idx):
            g = proj_steps(idx)
            next(g)          # emits the x-chunk DMA
            if idx == 0:
                init_rest()
            return g

        if trunc == "proj" or INTERLEAVE == "phase":
            for idx in range(NTC):
                drain(start_proj(idx))
            if trunc != "proj":
                for idx, (p, j) in enumerate(order):
                    if idx == 4:
                        nc.sync.dma_start(wosb[:], wout[:])
                        nc.sync.dma_start(bosb[:], bout[:])
                    drain(attend_steps(p, j))
        else:
            for k in upfront:
                drain(start_proj(k))
            for idx, (p, j) in enumerate(order):
                if idx == 4:
                    nc.sync.dma_start(wosb[:], wout[:])
                    nc.sync.dma_start(bosb[:], bout[:])
                pgs = [start_proj(k) for k in proj_plan.get(idx, [])]
                ag = attend_steps(p, j)
                if INTERLEAVE == "fill" and pgs:
                    # pace projection units into the attend yield points --
                    # one sits between each k-tile's score and AV matmuls,
                    # covering the exp latency with PE work
                    from itertools import chain
                    pchain = chain(*pgs)
                    ny = 8 * (j + 1)
                    np_est = 15 * len(pgs)
                    acc = 0
                    for i, _ in enumerate(ag):
                        want = ((i + 1) * np_est) // ny
                        while pchain is not None and acc < want:
                            if next(pchain, "end") == "end":
                                pchain = None
                                break
                            acc += 1
                    drain(pchain)
                    continue
                if INTERLEAVE == "hybrid":
                    # prime ACT with the first k-tiles before the proj block
                    for _ in range(2):
                        next(ag, None)
                for pg in pgs:
                    drain(pg)
                drain(ag)

        pipe.close()

        if trunc in ("proj", "attn"):
            with tc.tile_pool(name="dumo", bufs=1) as dpool:
                dm = dpool.tile([128, TQ], F32, tag="d")
                nc.vector.tensor_copy(dm[:], qts[0][0:128, 0:TQ].bitcast(F32))
                nc.sync.dma_start(out[0:128, 0:TQ], dm[:])
            return

        # ---- all-to-all (head-sharded -> t-sharded) ----
        if with_collective is True:
            nc.gpsimd.collective_compute(
                "AllToAll", mybir.AluOpType.bypass,
                replica_groups=[list(range(NCORES))],
                ins=[a2a_in[:]], outs=[a2a_out[:]])
        elif with_collective is False:
            nc.sync.dma_start(a2a_out[:], a2a_in[:])
        # else (None): timing mode — caller aliases a2a_out to a2a_in

        # ---- output projection (rows TSL per core) ----
        with (
            tc.tile_pool(name="yts", bufs=1) as ytspool,
            tc.tile_pool(name="pout", bufs=4, space="PSUM") as poutp,
            tc.tile_pool(name="osb", bufs=4) as osbpool,
        ):
            yts = ytspool.tile([128, NCT, TQ], ydt, tag="yts")
            for cc in range(NCT):
                nc.sync.dma_start(yts[:, cc, :], a2a_out[cc, :, :])

            for tt in range(TSL // 128):
                pos = [poutp.tile([128, TQ], F32, tag="pout",
                                  name=f"pos{tt}_{h}") for h in range(2)]
                for cc in range(NCT):
                    for n in range(2):
                        nc.tensor.matmul(
                            pos[n][:], yts[:, cc, 128 * tt:128 * (tt + 1)],
                            wosb[:, cc, TQ * n:TQ * (n + 1)],
                            start=(cc == 0), stop=(cc == NCT - 1))
                for n in range(2):
                    osb = osbpool.tile([128, TQ], F32, tag="osb")
                    nc.vector.tensor_add(osb[:], pos[n][:],
                                         bosb[:, TQ * n:TQ * (n + 1)])
                    nc.sync.dma_start(
                        out[128 * tt:128 * (tt + 1), TQ * n:TQ * (n + 1)],
                        osb[:])


def make_core_inputs(x, w_qkv, b_qkv, w_out, b_out):
    """Host-side shard/transform. Returns list of per-core input dicts."""
    x = np.asarray(x, np.float32)
    w_qkv = np.asarray(w_qkv, np.float32)
    b_qkv = np.asarray(b_qkv, np.float32)
    w_out = np.asarray(w_out, np.float32)
    b_out = np.asarray(b_out, np.float32)

    import ml_dtypes
    xdt = ml_dtypes.bfloat16 if XT_BF16 else np.float32
    ydt = ml_dtypes.bfloat16 if Y_BF16 else np.float32

    # softmax rows sum to 1, so the V bias contributes (b_v @ w_out) to
    # every output row — fold it into the output bias.
    b_eff = b_out + b_qkv[2 * C:] @ w_out.astype(ydt).astype(np.float32)
    bout_rep = np.ascontiguousarray(
        np.broadcast_to(b_eff.astype(np.float32), (128, C)))
    # chunk-contiguous x^T: xt[tc0, p, kc, q] = x_flat[TQ*tc0+q, 128*kc+p]
    xt = np.ascontiguousarray(
        x.reshape(NTC, TQ, NCT, 128).transpose(0, 3, 2, 1)).astype(xdt)
    # weight layouts matching the on-device SBUF tiles (contiguous DMAs)
    wout2 = np.ascontiguousarray(
        w_out.reshape(NCT, 128, C).transpose(1, 0, 2)).astype(ydt)
    vaones = np.ones((128, 2, NKK, 64), np.float32)
    zeros = np.zeros((64, T), np.float32)
    in_maps = []
    for c in range(NCORES):
        s = slice(128 * c, 128 * (c + 1))
        wq = w_qkv[:, :C][:, s] * 0.125
        wk = w_qkv[:, C:2 * C][:, s]
        wv = w_qkv[:, 2 * C:][:, s]
        wc = np.concatenate([wq, wk, wv], axis=1)
        wc2 = np.ascontiguousarray(
            wc.reshape(NCT, 128, 3, 128).transpose(1, 2, 0, 3)).astype(xdt)
        bc3 = np.ascontiguousarray(
            np.stack([b_qkv[:C][s] * 0.125, b_qkv[C:2 * C][s],
                      np.zeros(128, np.float32)], axis=1))
        in_maps.append({
            "xt": xt, "wqkv": wc2, "bqkv": bc3,
            "wout": wout2, "bout": bout_rep,
            "vaones": vaones, "zeros": zeros,
        })
    return in_maps


_NC_CACHE = {}


def kernel(x, w_qkv, b_qkv, w_out, b_out):
    in_maps = make_core_inputs(x, w_qkv, b_qkv, w_out, b_out)
    if "nc" not in _NC_CACHE:
        _NC_CACHE["nc"] = build()
    nc = _NC_CACHE["nc"]
    res = run_bass_kernel_spmd(nc, in_maps, core_ids=list(range(NCORES)))
    full = np.concatenate([res.results[c]["out"] for c in range(NCORES)],
                          axis=0)
    return full.reshape(B, T, C)
